# revision 3
# baseline (speedup 1.0000x reference)
"""KAN spline layer (B=16384, IN=512, OUT=1024, cubic B-splines, 8 coefs per
(in,out) pair) as a Bass/Tile kernel for 8 Trainium2 NeuronCores.

Strategy
--------
Data-parallel over batch (2048 rows/core), weights replicated.

Math: with t = (x - g0)/h - 3 in (0.71, 4.29), the 8 basis values are
plane_k(t) = K3(|t-(k-1)|), K3(d) = relu(2-d)^3 - 4*relu(1-d)^3 (the 1/6 is
folded into host-prepped weights).  y = silu(x) @ sb + planes @ w2.

Device pipeline per 512-column batch chunk:
 - planes k in {2..5} (need a true |.|): ScalarE Abs + Relu, then one fused
   custom-DVE op computes a^3 - 4*relu(a-1)^3 straight to fp8.
 - planes k in {0,1,6,7}: t-(k-1) has a fixed sign on the whole grid range, so
   d comes from one/two stock tensor_scalar ops (4x mode) and a second fused
   custom-DVE op evaluates the (sign-folded) plane from min(d,2)-2.
 - silu: one ScalarE Silu straight to fp8.
All 36 fp8 rows (32 plane slots + 4 silu slots) land pair-adjacent in one SBUF
tile, and the contraction runs as 18 fp8 DoubleRow matmuls (256-deep each) per
[128 x 512] PSUM half at 2x PE rate.  Host pre-scales weights by SC=64 (fp8
range) with stochastic rounding on the degenerate scale_base; the 1/SC rides
the PSUM->SBUF drain (ScalarE Identity, fp16 out), and y is upcast on host.
"""

import numpy as np
import ml_dtypes

import concourse.bass as bass
import concourse.mybir as mybir
import concourse.tile as tile
from concourse import bacc
from concourse.bass_utils import run_bass_kernel_spmd

F32 = mybir.dt.float32
F16 = mybir.dt.float16
F8 = mybir.dt.float8e4
ALU = mybir.AluOpType
AFT = mybir.ActivationFunctionType

N_CORES = 8
B_FULL = 16384
BS = B_FULL // N_CORES          # 2048 batch rows per core
IN_DIM = 512
OUT_DIM = 1024
NK = 8
NCH = IN_DIM // 128             # 4 in-dim chunks of 128 partitions
BCH = 512                       # batch columns per chunk
NBCH = BS // BCH                # 4 chunks per core
NSLOT = NCH * NK + NCH          # 32 plane slots + 4 silu slots
SC = 64.0                       # fp8 weight scale (undone in the drain)
DVE_K = (0, 1, 6, 7)            # planes with sign-fixed t-(k-1): DVE path, -w
ACT_K = (2, 3, 4, 5)            # planes needing |.|: ScalarE path, +w

# ---- custom DVE ops ---------------------------------------------------------
from concourse.dve_ops import DveOp, OPS, _SUB_OPCODE_FOR_NAME, _CUSTOM_DVE_ROW_BASE
from concourse.dve_spec import Spec, Src0, C0, C1, One, relu, sq, lower
from concourse.dve_uop import DveOpSpec


def _register(name, spec):
    if name in _SUB_OPCODE_FOR_NAME:
        return next(op for op in OPS if op.name == name)
    opcode = _CUSTOM_DVE_ROW_BASE + len(OPS)
    assert opcode < 0x20, "custom-DVE opcode table overflow"
    shas = {}
    for ver in ("v3", "v4"):
        try:
            s = DveOpSpec(name=name, opcode=opcode, uops=lower(spec, ver=ver),
                          rd1_en=False)
            shas[ver] = s.sha(ver)
        except Exception:
            pass
    op = DveOp(name, spec, subdim=False, uops_sha=shas)
    OPS.append(op)
    _SUB_OPCODE_FOR_NAME[name] = opcode
    return op


def _mk_opca():
    # in0 = a = relu(2-d) >= 0; out = a^3 - 4*relu(a-1)^3   (s1 = -4)
    e = Src0 - One
    b = relu(e)
    b3 = sq(b) * b
    m = b3 * C1
    a3 = sq(Src0) * Src0
    return Spec(body=a3 + m,
                reference=lambda in0, s0, s1: in0**3 + s1 * np.maximum(in0 - 1, 0)**3)


def _mk_opcn():
    # in0 = min(d,2)-2 = -a;  out = -(a^3 - 4b^3)   (s0 = -1, s1 = +4)
    e = C0 - Src0
    b = relu(e)
    b3 = sq(b) * b
    m = b3 * C1
    a3 = sq(Src0) * Src0
    return Spec(body=a3 + m,
                reference=lambda in0, s0, s1: in0**3 + s1 * np.maximum(s0 - in0, 0)**3)


OPCA = _register("KAN_PLANE_A", _mk_opca())
OPCN = _register("KAN_PLANE_N", _mk_opcn())


# ---- device kernel ----------------------------------------------------------
def kan_body(ctx, tc, y, t_d, x_d, w_d):
    nc = tc.nc

    consts = ctx.enter_context(tc.tile_pool(name="consts", bufs=1))
    io_pool = ctx.enter_context(tc.tile_pool(name="io", bufs=2))
    tmp_pool = ctx.enter_context(tc.tile_pool(name="tmps", bufs=1))
    pall_pool = ctx.enter_context(tc.tile_pool(name="pall", bufs=2))
    yout_pool = ctx.enter_context(tc.tile_pool(name="yout", bufs=2))
    ypsum = ctx.enter_context(tc.tile_pool(name="ypsum", bufs=3, space="PSUM"))

    wsb = consts.tile([128, NSLOT, OUT_DIM], F8)
    for q in range(4):
        s0 = q * (NSLOT // 4)
        nc.sync.dma_start(
            wsb[:, s0:s0 + NSLOT // 4, :],
            w_d[s0 * 128:(s0 + NSLOT // 4) * 128, :].rearrange(
                "(s p) o -> p s o", p=128))
    biasK = consts.tile([128, NK], F32)
    for k in range(NK):
        nc.vector.memset(biasK[:, k:k + 1], float(1 - k))
    bias2 = consts.tile([128, 1], F32)
    nc.vector.memset(bias2, 2.0)
    sc_dr = consts.tile([128, 1], F32)
    nc.vector.memset(sc_dr, 1.0 / SC)

    FD = NCH * BCH
    for bc in range(NBCH):
        b0 = bc * BCH

        tt = io_pool.tile([128, NCH, BCH], F16)
        nc.sync.dma_start(tt, t_d[:, b0:b0 + BCH].rearrange("(c p) b -> p c b", p=128))
        xx = io_pool.tile([128, NCH, BCH], F16)
        nc.sync.dma_start(xx, x_d[:, b0:b0 + BCH].rearrange("(c p) b -> p c b", p=128))
        tflat = tt.rearrange("p c b -> p (c b)")

        pall = pall_pool.tile([128, NSLOT, BCH], F8)
        pk = pall[:, :NCH * NK, :].rearrange("p (c k) b -> p c k b", k=NK)

        # silu slots (fp8 straight from ScalarE)
        nc.scalar.activation(
            pall[:, NCH * NK:, :].rearrange("p c b -> p (c b)"),
            xx.rearrange("p c b -> p (c b)"), AFT.Silu)

        for k in range(NK):
            oslots = pk[:, :, k, :]
            if k in ACT_K:
                dd = tmp_pool.tile([128, FD], F16, tag="da", name="da", bufs=2)
                nc.scalar.activation(dd, tflat, AFT.Abs,
                                     bias=biasK[:, k:k + 1], scale=1.0)
                aa = tmp_pool.tile([128, FD], F16, tag="aa", name="aa", bufs=2)
                nc.scalar.activation(aa, dd, AFT.Relu, bias=bias2, scale=-1.0)
                nc.vector._custom_dve(
                    OPCA, out=oslots,
                    in0=aa.rearrange("p (c b) -> p c b", c=NCH),
                    s0=0.0, s1=-4.0)
            else:
                # d = +-(t-(k-1)) is nonneg on the whole input range
                if k == 1:
                    dd = tflat
                else:
                    dd = tmp_pool.tile([128, FD], F16, tag="dv", name="dv", bufs=2)
                    if k == 0:
                        nc.vector.tensor_scalar(dd, tflat, float(k - 1), 0.0,
                                                ALU.subtract, ALU.add)
                    else:
                        nc.vector.tensor_scalar(dd, tflat, -1.0, float(k - 1),
                                                ALU.mult, ALU.add)
                am = tmp_pool.tile([128, FD], F16, tag="am", name="am", bufs=2)
                nc.vector.tensor_scalar(am, dd, 2.0, 2.0, ALU.min, ALU.subtract)
                nc.vector._custom_dve(
                    OPCN, out=oslots,
                    in0=am.rearrange("p (c b) -> p c b", c=NCH),
                    s0=-1.0, s1=4.0)

        # ---- 18 fp8 DoubleRow matmuls per [128, 512] psum half -------------
        for bt in range(BCH // 128):
            ps = ypsum.tile([128, OUT_DIM], F32)
            bcol = slice(bt * 128, (bt + 1) * 128)
            for oh in range(2):
                o0 = oh * 512
                for j in range(NSLOT // 2):
                    nc.tensor.matmul(
                        ps[:, o0:o0 + 512],
                        pall[:, 2 * j:2 * j + 2, bcol],
                        wsb[:, 2 * j:2 * j + 2, o0:o0 + 512],
                        start=(j == 0), stop=(j == NSLOT // 2 - 1),
                        perf_mode=mybir.MatmulPerfMode.DoubleRow)
            yt = yout_pool.tile([128, OUT_DIM], F16)
            nc.scalar.activation(yt, ps, AFT.Identity, bias=0.0, scale=sc_dr)
            nc.sync.dma_start(y[b0 + bt * 128: b0 + (bt + 1) * 128, :], yt)


def build_nc(bs=BS):
    from contextlib import ExitStack

    nc = bacc.Bacc("TRN2", target_bir_lowering=False, debug=False)
    t_d = nc.dram_tensor("t", [IN_DIM, bs], F16, kind="ExternalInput").ap()
    x_d = nc.dram_tensor("x", [IN_DIM, bs], F16, kind="ExternalInput").ap()
    w_d = nc.dram_tensor("w", [NSLOT * 128, OUT_DIM], F8, kind="ExternalInput").ap()
    y = nc.dram_tensor("y", [bs, OUT_DIM], F16, kind="ExternalOutput").ap()
    with tile.TileContext(nc) as tc:
        with ExitStack() as ctx:
            kan_body(ctx, tc, y, t_d, x_d, w_d)
    nc.compile()
    return nc


# ---- host prep --------------------------------------------------------------
def _sr_e4m3(v, rng):
    """Unbiased stochastic rounding to fp8 e4m3 (kills the correlated bias of
    round-to-nearest on degenerate constant weights like scale_base)."""
    F8n = ml_dtypes.float8_e4m3fn
    lo = v.astype(F8n).astype(np.float32)
    resid = v - lo
    eps = np.maximum(np.abs(lo) * 2.0**-3, 2.0**-9).astype(np.float32)
    hi = np.where(resid > 0, lo + eps, lo - eps).astype(F8n).astype(np.float32)
    p = np.where(hi != lo, np.abs(resid) / np.maximum(np.abs(hi - lo), 1e-30), 0.0)
    out = np.where(rng.rand(*v.shape) < p, hi, lo)
    return out.astype(F8n)


def host_prep(x, grid, coef, scale_base):
    F8n = ml_dtypes.float8_e4m3fn
    x = np.asarray(x, dtype=np.float32)
    grid = np.asarray(grid, dtype=np.float32)
    coef = np.asarray(coef, dtype=np.float32)
    scale_base = np.asarray(scale_base, dtype=np.float32)

    g0 = grid[:, 0]
    h = (grid[:, -1] - grid[:, 0]) / np.float32(grid.shape[1] - 1)
    tsc = (1.0 / h).astype(np.float32)
    tbi = (-g0 / h - 3.0).astype(np.float32)

    tT = np.ascontiguousarray((x * tsc[None, :] + tbi[None, :]).T.astype(np.float16))
    xT = np.ascontiguousarray(x.T.astype(np.float16))

    w2 = np.transpose(coef, (2, 0, 1)) / 6.0  # [k, in, out]
    wq = np.empty((NSLOT * 128, OUT_DIM), np.float32)
    for s in range(NCH * NK):
        c, k = divmod(s, NK)
        sign = -1.0 if k in DVE_K else 1.0
        wq[s * 128:(s + 1) * 128] = w2[k, c * 128:(c + 1) * 128] * (SC * sign)
    rng = np.random.RandomState(0xC0FFEE)
    sb_q = _sr_e4m3(scale_base * SC, rng).astype(np.float32)
    for c in range(NCH):
        wq[(NCH * NK + c) * 128:(NCH * NK + c + 1) * 128] = \
            sb_q[c * 128:(c + 1) * 128]
    w8 = wq.astype(F8n)
    return tT, xT, np.ascontiguousarray(w8).view(np.uint8)


_NC_CACHE = {}


def get_nc(bs=BS):
    if bs not in _NC_CACHE:
        _NC_CACHE[bs] = build_nc(bs)
    return _NC_CACHE[bs]


def make_in_maps(x, grid, coef, scale_base):
    tT, xT, w8 = host_prep(x, grid, coef, scale_base)
    return [
        {"t": np.ascontiguousarray(tT[:, c * BS:(c + 1) * BS]),
         "x": np.ascontiguousarray(xT[:, c * BS:(c + 1) * BS]),
         "w": w8}
        for c in range(N_CORES)
    ]


def kernel(x, grid, coef, scale_base):
    nc = get_nc(BS)
    in_maps = make_in_maps(x, grid, coef, scale_base)
    res = run_bass_kernel_spmd(nc, in_maps, core_ids=list(range(N_CORES)))
    return np.concatenate(
        [np.asarray(res.results[c]["y"]).astype(np.float32) for c in range(N_CORES)],
        axis=0)


# revision 30
# speedup vs baseline: 1.1658x; 1.1658x over previous
"""KAN spline layer (B=16384, IN=512, OUT=1024, cubic B-splines, 8 coefs per
(in,out) pair) as a Bass/Tile kernel for 8 Trainium2 NeuronCores.

Strategy
--------
Data-parallel over batch (2048 rows/core), weights replicated.

Math: with t = (x - g0)/h - 3 in (0.71, 4.29), the 8 basis values are
plane_k(t) = K3(|t-(k-1)|), K3(d) = relu(2-d)^3 - 4*relu(1-d)^3 (the 1/6 is
folded into host-prepped weights).  y = silu(x) @ sb + planes @ w2.

Device pipeline per 512-column batch chunk:
 - planes k in {2..5}: ScalarE Abs + Relu, then one fused custom-DVE op
   computes a^3 - 4*relu(a-1)^3 straight to fp8.
 - planes k in {0,1,6,7}: t-(k-1) has a fixed sign on the whole grid range, so
   d comes from stock tensor_scalar ops (4x mode) / ScalarE, and a second
   fused custom-DVE op evaluates the sign-folded plane from min(d,2)-2.
 - silu: one ScalarE Silu straight to fp8.
All 36 fp8 rows (32 plane slots + 4 silu slots) land pair-adjacent in one SBUF
tile, ordered so earlier-finishing planes occupy lower slots; the contraction
runs as 18 fp8 DoubleRow matmuls (256-deep each) per [128 x 512] PSUM half at
2x PE rate, issued in slot order so the PE starts while later planes are still
being computed.  Host pre-scales weights by SC=64 (fp8 range) with stochastic
rounding on the degenerate scale_base; the 1/SC rides the PSUM->SBUF drain
(GpSimd steady-state, ScalarE/DVE for the last chunk), and y (fp16) is upcast
on host.
"""

import numpy as np
import ml_dtypes

import concourse.bass as bass
import concourse.mybir as mybir
import concourse.tile as tile
from concourse import bacc
from concourse.bass_utils import run_bass_kernel_spmd

F32 = mybir.dt.float32
F16 = mybir.dt.float16
F8 = mybir.dt.float8e4
ALU = mybir.AluOpType
AFT = mybir.ActivationFunctionType

N_CORES = 8
B_FULL = 16384
BS = B_FULL // N_CORES          # 2048 batch rows per core
IN_DIM = 512
OUT_DIM = 1024
NK = 8
NCH = IN_DIM // 128             # 4 in-dim chunks of 128 partitions
BCH = 512                       # batch columns per chunk
NBCH = BS // BCH                # 4 chunks per core
NSLOT = NCH * NK + NCH          # 32 plane slots + 4 silu slots
NPAIR = NSLOT // 2
SC = 64.0                       # fp8 weight scale (undone in the drain)
OPCN_K = (0, 1, 6, 7)           # planes evaluated by OPCN (sign-folded, -w)

# slot layout (pair-adjacent, readiness-ordered):
#   pairs 0,1:  silu (c0,c1),(c2,c3) -> slots c
#   pairs 2-5:  (k0, k1) per c     -> slots 4+2c / 5+2c
#   pairs 6-9:  (k2, k3) per c     -> slots 12+2c / 13+2c
#   pairs 10-13:(k6, k7) per c     -> slots 20+2c / 21+2c
#   pairs 14-17:(k4, k5) per c     -> slots 28+2c / 29+2c
_SEG_BASE = {0: 4, 1: 12, 3: 20, 2: 28}          # k//2 -> slot base


def slot_of(c, k):
    return _SEG_BASE[k // 2] + 2 * c + (k & 1)


def silu_slot(c):
    return c


# ---- custom DVE ops ---------------------------------------------------------
from concourse.dve_ops import DveOp, OPS, _SUB_OPCODE_FOR_NAME, _CUSTOM_DVE_ROW_BASE
from concourse.dve_spec import Spec, Src0, C0, C1, One, relu, sq, lower
from concourse.dve_uop import DveOpSpec


def _register(name, spec):
    if name in _SUB_OPCODE_FOR_NAME:
        return next(op for op in OPS if op.name == name)
    opcode = _CUSTOM_DVE_ROW_BASE + len(OPS)
    assert opcode < 0x20, "custom-DVE opcode table overflow"
    shas = {}
    for ver in ("v3", "v4"):
        try:
            s = DveOpSpec(name=name, opcode=opcode, uops=lower(spec, ver=ver),
                          rd1_en=False)
            shas[ver] = s.sha(ver)
        except Exception:
            pass
    op = DveOp(name, spec, subdim=False, uops_sha=shas)
    OPS.append(op)
    _SUB_OPCODE_FOR_NAME[name] = opcode
    return op


def _mk_opca():
    # in0 = a = relu(2-d) >= 0; out = a^3 - 4*relu(a-1)^3   (s1 = -4)
    e = Src0 - One
    b = relu(e)
    b3 = sq(b) * b
    m = b3 * C1
    a3 = sq(Src0) * Src0
    return Spec(body=a3 + m,
                reference=lambda in0, s0, s1: in0**3 + s1 * np.maximum(in0 - 1, 0)**3)


def _mk_opcn():
    # in0 = min(d,2)-2 = -a;  out = -(a^3 - 4b^3)   (s0 = -1, s1 = +4)
    e = C0 - Src0
    b = relu(e)
    b3 = sq(b) * b
    m = b3 * C1
    a3 = sq(Src0) * Src0
    return Spec(body=a3 + m,
                reference=lambda in0, s0, s1: in0**3 + s1 * np.maximum(s0 - in0, 0)**3)


OPCA = _register("KAN_PLANE_A", _mk_opca())
OPCN = _register("KAN_PLANE_N", _mk_opcn())


# ---- device kernel ----------------------------------------------------------
def kan_body(ctx, tc, y, t_d, x_d, w_d):
    nc = tc.nc

    consts = ctx.enter_context(tc.tile_pool(name="consts", bufs=1))
    io_pool = ctx.enter_context(tc.tile_pool(name="io", bufs=3))
    tmp_pool = ctx.enter_context(tc.tile_pool(name="tmps", bufs=1))
    pall_pool = ctx.enter_context(tc.tile_pool(name="pall", bufs=3))
    yout_pool = ctx.enter_context(tc.tile_pool(name="yout", bufs=2))
    ypsum = ctx.enter_context(tc.tile_pool(name="ypsum", bufs=1, space="PSUM"))

    # first chunk's inputs lead the SP DMA queue; weights ride the gpsimd
    # SWDGE queue, sliced in slot order so early matmul pairs unblock first.
    tt0 = io_pool.tile([128, NCH, BCH], F16, tag="tt", name="tt0")
    nc.sync.dma_start(tt0, t_d[:, 0:BCH].rearrange("(c p) b -> p c b", p=128))
    xx0 = io_pool.tile([128, NCH, BCH], F16, tag="xx", name="xx0")
    nc.sync.dma_start(xx0, x_d[:, 0:BCH].rearrange("(c p) b -> p c b", p=128))

    wsb = consts.tile([128, NSLOT, OUT_DIM], F8)
    for q in range(6):
        s0 = q * 6
        nc.sync.dma_start(
            wsb[:, s0:s0 + 6, :],
            w_d[s0 * 128:(s0 + 6) * 128, :].rearrange("(s p) o -> p s o", p=128))
    biasK = consts.tile([128, NK], F32)
    for k in range(NK):
        nc.vector.memset(biasK[:, k:k + 1], float(1 - k))
    bias2 = consts.tile([128, 1], F32)
    nc.vector.memset(bias2, 2.0)
    sc_dr = consts.tile([128, 1], F32)
    nc.vector.memset(sc_dr, 1.0 / SC)

    FD = NCH * BCH
    pending = []                # (pss, b0, on_act) drains deferred one chunk
    for bc in range(NBCH):
        b0 = bc * BCH

        if bc == 0:
            tt, xx = tt0, xx0
        else:
            tt = io_pool.tile([128, NCH, BCH], F16, tag="tt", name=f"tt{bc}")
            nc.sync.dma_start(tt, t_d[:, b0:b0 + BCH].rearrange("(c p) b -> p c b", p=128))
            xx = io_pool.tile([128, NCH, BCH], F16, tag="xx", name=f"xx{bc}")
            nc.sync.dma_start(xx, x_d[:, b0:b0 + BCH].rearrange("(c p) b -> p c b", p=128))
        tflat = tt.rearrange("p c b -> p (c b)")

        # previous chunk's PSUM drains first: the in-order engine queues must
        # not head-of-line-block them behind this chunk's elementwise work
        for args in pending:
            _drain(nc, sc_dr, y, *args)
        pending = []

        pall = pall_pool.tile([128, NSLOT, BCH], F8)

        def plane_out(k):
            base = _SEG_BASE[k // 2]
            seg = pall[:, base:base + 8, :]
            return seg.rearrange("p (c two) b -> p two c b", two=2)[:, k & 1, :, :]

        def ts_d(k, on_pool=False):
            # d = +-(t-(k-1)), sign-definite
            dd = tmp_pool.tile([128, FD], F16, tag="dv", name="dv", bufs=3)
            eng = nc.gpsimd if on_pool else nc.vector
            if k < 2:
                eng.tensor_scalar(dd, tflat, float(k - 1), 0.0,
                                  ALU.subtract, ALU.add)
            else:
                eng.tensor_scalar(dd, tflat, -1.0, float(k - 1),
                                  ALU.mult, ALU.add)
            return dd

        def ts_am(dd, on_pool=False):
            am = tmp_pool.tile([128, FD], F16, tag="am", name="am", bufs=3)
            if on_pool:
                nc.gpsimd.tensor_scalar(am, dd, 2.0, 2.0, ALU.min, ALU.subtract)
            else:
                nc.vector.tensor_scalar(am, dd, 2.0, 2.0, ALU.min, ALU.subtract)
            return am

        def opcn(k, am):
            nc.vector._custom_dve(
                OPCN, out=plane_out(k),
                in0=am.rearrange("p (c b) -> p c b", c=NCH), s0=-1.0, s1=4.0)

        def act_da(k):
            dd = tmp_pool.tile([128, FD], F16, tag="da", name="da", bufs=3)
            nc.scalar.activation(dd, tflat, AFT.Abs,
                                 bias=biasK[:, k:k + 1], scale=1.0)
            aa = tmp_pool.tile([128, FD], F16, tag=f"aa{k % 2}",
                               name=f"aa{k % 2}", bufs=3)
            nc.scalar.activation(aa, dd, AFT.Relu, bias=bias2, scale=-1.0)
            return aa

        def opca(k, aa):
            nc.vector._custom_dve(
                OPCA, out=plane_out(k),
                in0=aa.rearrange("p (c b) -> p c b", c=NCH), s0=0.0, s1=-4.0)

        # Stagger per-engine queues so plane pairs complete in slot order.
        nc.scalar.activation(                       # silu slots, fp8 out
            pall[:, 0:NCH, :].rearrange("p c b -> p (c b)"),
            xx.rearrange("p c b -> p (c b)"), AFT.Silu)
        am0 = ts_am(ts_d(0), on_pool=True)
        am1 = ts_am(tflat, on_pool=True)
        opcn(0, am0)
        opcn(1, am1)
        a2 = act_da(2)
        opca(2, a2)
        a3 = act_da(3)
        opca(3, a3)
        opcn(6, ts_am(ts_d(6, on_pool=True), on_pool=True))
        opcn(7, ts_am(ts_d(7)))
        a4 = act_da(4)
        opca(4, a4)
        a5 = act_da(5)
        opca(5, a5)

        # previous chunk's PSUM drains (deferred so the in-order engine queues
        # never head-of-line-block the next chunk's elementwise work)
        # ---- 18 fp8 DoubleRow matmuls per [128, 512] psum half -------------
        pss = [ypsum.tile([128, OUT_DIM], F32, tag=f"ps{bt}", name=f"ps{bt}",
                          bufs=1)
               for bt in range(BCH // 128)]
        for j in range(NPAIR):
            for bt in range(BCH // 128):
                bcol = slice(bt * 128, (bt + 1) * 128)
                for oh in range(2):
                    o0 = oh * 512
                    nc.tensor.matmul(
                        pss[bt][:, o0:o0 + 512],
                        pall[:, 2 * j:2 * j + 2, bcol],
                        wsb[:, 2 * j:2 * j + 2, o0:o0 + 512],
                        start=(j == 0), stop=(j == NPAIR - 1),
                        perf_mode=mybir.MatmulPerfMode.DoubleRow)
        pending.append((yout_pool, pss, b0, bc == NBCH - 1))
    for args in pending:
        _drain(nc, sc_dr, y, *args)


def _drain(nc, sc_dr, y, yout_pool, pss, b0, on_act):
    for bt in range(len(pss)):
        yt = yout_pool.tile([128, OUT_DIM], F16, tag=f"yt{bt}", name=f"yt{bt}",
                            bufs=2)
        if bt != 3:
            nc.scalar.activation(yt, pss[bt], AFT.Identity, bias=0.0,
                                 scale=sc_dr)
        else:
            nc.vector.tensor_scalar(yt, pss[bt], 1.0 / SC, 0.0,
                                    ALU.mult, ALU.add)
        nc.scalar.dma_start(y[b0 + bt * 128: b0 + (bt + 1) * 128, :], yt)


def build_nc(bs=BS):
    from contextlib import ExitStack

    nc = bacc.Bacc("TRN2", target_bir_lowering=False, debug=False)
    t_d = nc.dram_tensor("t", [IN_DIM, bs], F16, kind="ExternalInput").ap()
    x_d = nc.dram_tensor("x", [IN_DIM, bs], F16, kind="ExternalInput").ap()
    w_d = nc.dram_tensor("w", [NSLOT * 128, OUT_DIM], F8, kind="ExternalInput").ap()
    y = nc.dram_tensor("y", [bs, OUT_DIM], F16, kind="ExternalOutput").ap()
    with tile.TileContext(nc) as tc:
        with ExitStack() as ctx:
            kan_body(ctx, tc, y, t_d, x_d, w_d)
    nc.compile()
    return nc


# ---- host prep --------------------------------------------------------------
def _sr_e4m3(v, rng):
    """Unbiased stochastic rounding to fp8 e4m3 (kills the correlated bias of
    round-to-nearest on degenerate constant weights like scale_base)."""
    F8n = ml_dtypes.float8_e4m3fn
    lo = v.astype(F8n).astype(np.float32)
    resid = v - lo
    eps = np.maximum(np.abs(lo) * 2.0**-3, 2.0**-9).astype(np.float32)
    hi = np.where(resid > 0, lo + eps, lo - eps).astype(F8n).astype(np.float32)
    p = np.where(hi != lo, np.abs(resid) / np.maximum(np.abs(hi - lo), 1e-30), 0.0)
    out = np.where(rng.rand(*v.shape) < p, hi, lo)
    return out.astype(F8n)


def host_prep(x, grid, coef, scale_base):
    F8n = ml_dtypes.float8_e4m3fn
    x = np.asarray(x, dtype=np.float32)
    grid = np.asarray(grid, dtype=np.float32)
    coef = np.asarray(coef, dtype=np.float32)
    scale_base = np.asarray(scale_base, dtype=np.float32)

    g0 = grid[:, 0]
    h = (grid[:, -1] - grid[:, 0]) / np.float32(grid.shape[1] - 1)
    tsc = (1.0 / h).astype(np.float32)
    tbi = (-g0 / h - 3.0).astype(np.float32)

    tT = np.ascontiguousarray((x * tsc[None, :] + tbi[None, :]).T.astype(np.float16))
    xT = np.ascontiguousarray(x.T.astype(np.float16))

    w2 = np.transpose(coef, (2, 0, 1)) / 6.0  # [k, in, out]
    rng = np.random.RandomState(0xC0FFEE)
    sb_q = _sr_e4m3(scale_base * SC, rng).astype(np.float32)

    wq = np.empty((NSLOT * 128, OUT_DIM), np.float32)
    for c in range(NCH):
        for k in range(NK):
            s = slot_of(c, k)
            sign = -1.0 if k in OPCN_K else 1.0
            wq[s * 128:(s + 1) * 128] = w2[k, c * 128:(c + 1) * 128] * (SC * sign)
        s = silu_slot(c)
        wq[s * 128:(s + 1) * 128] = sb_q[c * 128:(c + 1) * 128]
    w8 = wq.astype(F8n)
    return tT, xT, np.ascontiguousarray(w8).view(np.uint8)


_NC_CACHE = {}


def get_nc(bs=BS):
    if bs not in _NC_CACHE:
        _NC_CACHE[bs] = build_nc(bs)
    return _NC_CACHE[bs]


def make_in_maps(x, grid, coef, scale_base):
    tT, xT, w8 = host_prep(x, grid, coef, scale_base)
    return [
        {"t": np.ascontiguousarray(tT[:, c * BS:(c + 1) * BS]),
         "x": np.ascontiguousarray(xT[:, c * BS:(c + 1) * BS]),
         "w": w8}
        for c in range(N_CORES)
    ]


def kernel(x, grid, coef, scale_base):
    nc = get_nc(BS)
    in_maps = make_in_maps(x, grid, coef, scale_base)
    res = run_bass_kernel_spmd(nc, in_maps, core_ids=list(range(N_CORES)))
    return np.concatenate(
        [np.asarray(res.results[c]["y"]).astype(np.float32) for c in range(N_CORES)],
        axis=0)
